# revision 21
# baseline (speedup 1.0000x reference)
"""Trainium2 Bass kernel for nn_Attention_45183055954094.

Cosine-similarity attention (temp=30) over 64 independent instances of
1024 tokens x 128 channels, shared QK projection to head dim 32,
residual, InstanceL2Norm. Data-parallel: 8 instances per NeuronCore.

v2 design (vs. the 224us baseline):
  - Norm phase packed: up to 4 instances live in the partition
    32-blocks of one [128,1024] tile; projections are col-tiled
    matmuls; the |q+b|^2 norm sums come from blk@q^2 + blkb@q (bias
    folded into a second accumulating matmul) so no ACT Square is
    needed; 1/sqrt via exp(-0.5*ln(x)) so the ONLY ACT table set used
    anywhere is natural_log_exp_and_others (no table thrash, norm
    phases interleave freely with exp phases).
  - Qhat/Khat stored bf16: S^T = K^T Q per (instance, j-tile) is a
    single N=1024 bf16 matmul; the two instances of a pair run on
    different PE row-groups (tile_position) concurrently.
  - E = exp(30 S) stored bf16; AV matmuls stream bf16 at N=1024.
  - Z = colsum(E) via a DVE bf16 add-tree (7 adds) + ONE ones-matmul
    instead of 8 ones-matmuls: moves ~24us/core from PE to DVE.
  - f32->f32r via bitcast (no DVE CAST passes).
  - Software pipeline: deferred z/av/bz matmuls + DVE tails of pair
    p-1 drain inside pair p's exp stream (issued BEFORE the next
    blocking S matmul) so the PE never idles into HAM re-throttle.
"""

import sys

for _p in ("/opt/trn_rl_repo", "/root/.axon_site/_ro/trn_rl_repo"):
    if _p not in sys.path:
        sys.path.insert(0, _p)

import numpy as np

B, N, C, H, W = 16, 4, 128, 32, 32
HW = H * W           # 1024 tokens
NI = B * N           # 64 instances
NCORES = 8
IPC = NI // NCORES   # 8 instances per core

_CACHE = {}


def _build(ipc=IPC, debug=False, stop_after=None):
    import contextlib

    import concourse.bass as bass
    import concourse.tile as tile
    from concourse import bacc, mybir
    from concourse.bass import ts

    f32 = mybir.dt.float32
    f32r = mybir.dt.float32r
    bf16 = mybir.dt.bfloat16
    AF = mybir.ActivationFunctionType

    nc = bacc.Bacc("TRN2", target_bir_lowering=False, debug=False)

    f1_d = nc.dram_tensor("f1", [ipc, C, HW], f32r, kind="ExternalInput").ap()
    f2_d = nc.dram_tensor("f2", [ipc, C, HW], f32r, kind="ExternalInput").ap()
    wt_d = nc.dram_tensor("wt", [C, C], f32r, kind="ExternalInput").ap()
    blk_d = nc.dram_tensor("blk", [C, C], f32, kind="ExternalInput").ap()
    bb1_d = nc.dram_tensor("bb1", [C, 1], f32, kind="ExternalInput").ap()
    bb2_d = nc.dram_tensor("bb2", [C, 1], f32, kind="ExternalInput").ap()
    id_d = nc.dram_tensor("ident", [C, C], f32r, kind="ExternalInput").ap()
    b1_d = nc.dram_tensor("b1", [C, 1], f32, kind="ExternalInput").ap()
    b2_d = nc.dram_tensor("b2", [C, 1], f32, kind="ExternalInput").ap()
    nb1_d = nc.dram_tensor("nb1", [C, 1], f32, kind="ExternalInput").ap()
    nb2_d = nc.dram_tensor("nb2", [C, 1], f32, kind="ExternalInput").ap()
    t2_d = nc.dram_tensor("t2x2", [C, 1], f32, kind="ExternalInput").ap()
    out_d = nc.dram_tensor("out", [ipc, C, HW], f32, kind="ExternalOutput").ap()
    if debug:
        dq_d = nc.dram_tensor("dbg_qtn", [2, C, HW], f32,
                              kind="ExternalOutput").ap()
        dk_d = nc.dram_tensor("dbg_ktn", [2, C, HW], f32,
                              kind="ExternalOutput").ap()
        dz_d = nc.dram_tensor("dbg_z", [ipc, 1, HW], f32,
                              kind="ExternalOutput").ap()
        dr_d = nc.dram_tensor("dbg_r", [ipc, C, HW], f32,
                              kind="ExternalOutput").ap()
        dp_d = nc.dram_tensor("dbg_pqs", [2, C, HW], f32,
                              kind="ExternalOutput").ap()
        dn_d = nc.dram_tensor("dbg_nsq", [2, C, HW], f32,
                              kind="ExternalOutput").ap()

    dbg = {"on": debug}

    with tile.TileContext(nc) as tc:
        with contextlib.ExitStack() as ctx:
            consts = ctx.enter_context(tc.tile_pool(name="consts", bufs=1))
            f1p = ctx.enter_context(tc.tile_pool(name="f1p", bufs=4))
            f2p = ctx.enter_context(tc.tile_pool(name="f2p", bufs=8))
            sqp = ctx.enter_context(tc.tile_pool(name="sqp", bufs=2))
            pqsp = ctx.enter_context(tc.tile_pool(name="pqsp", bufs=2))
            lep = ctx.enter_context(tc.tile_pool(name="lep", bufs=3))
            qkp = ctx.enter_context(tc.tile_pool(name="qkp", bufs=4))
            x2tp = ctx.enter_context(tc.tile_pool(name="x2tp", bufs=4))
            ep = ctx.enter_context(tc.tile_pool(name="ep", bufs=24))
            # bufs=5: with 4, the 5th tree tile (s03) would reuse the
            # slot of s01 which s03 itself reads -> ring self-deadlock
            ztp = ctx.enter_context(tc.tile_pool(name="ztp", bufs=5))
            zsp = ctx.enter_context(tc.tile_pool(name="zsp", bufs=2))
            bzp = ctx.enter_context(tc.tile_pool(name="bzp", bufs=2))
            t1p = ctx.enter_context(tc.tile_pool(name="t1p", bufs=2))
            rp = ctx.enter_context(tc.tile_pool(name="rp", bufs=4))
            scrp = ctx.enter_context(tc.tile_pool(name="scrp", bufs=1))
            colp = ctx.enter_context(tc.tile_pool(name="colp", bufs=12))
            gp = ctx.enter_context(tc.tile_pool(name="gp", bufs=4))
            op = ctx.enter_context(tc.tile_pool(name="op", bufs=2))
            ps = ctx.enter_context(tc.tile_pool(name="ps", bufs=1,
                                                space="PSUM"))

            # ---- constants (f32r via bitcast, no conversion copies) ----
            wt_sb = consts.tile([C, C], f32r, tag="wt")
            nc.sync.dma_start(wt_sb[:], wt_d[:])
            blk_sb = consts.tile([C, C], f32, tag="blk")
            nc.sync.dma_start(blk_sb[:], blk_d[:])
            bb1_sb = consts.tile([C, 1], f32, tag="bb1")
            nc.sync.dma_start(bb1_sb[:], bb1_d[:])
            bb2_sb = consts.tile([C, 1], f32, tag="bb2")
            nc.sync.dma_start(bb2_sb[:], bb2_d[:])
            blk_bf = consts.tile([C, C], bf16, tag="blkbf")
            nc.vector.tensor_copy(blk_bf[:], blk_sb[:])
            id_sb = consts.tile([C, C], f32r, tag="id")
            nc.sync.dma_start(id_sb[:], id_d[:])
            b1_sb = consts.tile([C, 1], f32, tag="b1")
            nc.sync.dma_start(b1_sb[:], b1_d[:])
            b2_sb = consts.tile([C, 1], f32, tag="b2")
            nc.sync.dma_start(b2_sb[:], b2_d[:])
            nb1_sb = consts.tile([C, 1], f32, tag="nb1")
            nc.sync.dma_start(nb1_sb[:], nb1_d[:])
            nb2_sb = consts.tile([C, 1], f32, tag="nb2")
            nc.sync.dma_start(nb2_sb[:], nb2_d[:])
            t2_sb = consts.tile([C, 1], f32, tag="t2")
            nc.sync.dma_start(t2_sb[:], t2_d[:])

            ones_bf = consts.tile([C, 1], bf16, tag="onesbf")
            nc.vector.memset(ones_bf[:], 1.0)
            ones_f = consts.tile([C, 1], f32, tag="onesf")
            nc.vector.memset(ones_f[:], 1.0)
            onesrow_f = consts.tile([1, C], f32, tag="onesrowf")
            nc.vector.memset(onesrow_f[:], 1.0)
            gbias_sb = consts.tile([1, 1], f32, tag="gbias")
            nc.vector.memset(gbias_sb[:], 1e-5 / 64.0)

            onesrow_rt = consts.tile([1, C], f32r, tag="onesrowr")
            nc.vector.tensor_copy(onesrow_rt[:], onesrow_f[:])
            wt_r = wt_sb[:]
            blk_r = blk_bf[:]
            id_r = id_sb[:]
            onesrow_r = onesrow_rt[:]

            qtn = {}     # quad -> [C, HW] bf16
            ktn = {}
            f1_sbs = {}
            f2_sbs = {}
            x2t_sbs = {}
            e_tiles = {}
            r_sbs = {}
            ssq_cols = {}
            zsb = {}

            pe_fifo = []

            def drain(units):
                u = 0
                while pe_fifo and u < units:
                    cost, fn = pe_fifo.pop(0)
                    fn()
                    u += cost

            # ---------- norm phase stage closures ----------
            def norm_stages(insts, quad, r0):
                """Project+normalize len(insts) instances into rows
                [r0 : r0+32n] of quad's qtn/ktn. Returns named stages."""
                n = len(insts)
                rw = slice(r0, r0 + 32 * n)

                box = {}

                def qk_tiles():
                    if quad not in qtn:
                        qtn[quad] = qkp.tile([C, HW], bf16, tag="qk",
                                             name=f"qtn{quad}")
                        ktn[quad] = qkp.tile([C, HW], bf16, tag="qk",
                                             name=f"ktn{quad}")
                    return qtn[quad], ktn[quad]

                def dma_f1():
                    qk_tiles()
                    for i in insts:
                        fsb = f1p.tile([C, HW], f32r, tag="f1",
                                       name=f"f1_{i}")
                        for h in range(2):
                            nc.sync.dma_start(fsb[:, ts(h, 512)],
                                              f1_d[i, :, ts(h, 512)])
                        f1_sbs[i] = fsb

                def dma_f2():
                    for i in insts:
                        f2sb = f2p.tile([C, HW], f32r, tag="f2",
                                        name=f"f2_{i}")
                        for h in range(2):
                            nc.sync.dma_start(f2sb[:, ts(h, 512)],
                                              f2_d[i, :, ts(h, 512)])
                        f2_sbs[i] = f2sb

                def mk_proj(which):
                    def s_proj():
                        # f32r matmuls cannot target offset dst partitions,
                        # so project per-instance with the 4x-replicated
                        # weight (full M=128, offset-0 dst, 1-bank halves)
                        # and DVE-copy the instance's replica row-block
                        # into the packed bf16 pqs tile.
                        pqs = pqsp.tile([C, HW], bf16, tag="pqs",
                                        name=f"pqs{which}{quad}_{r0}")
                        for a, i in enumerate(insts):
                            src = f1_sbs.pop(i) if which == "q" else f2_sbs[i]
                            fr = src[:]
                            ro = r0 + 32 * a
                            rs = slice(ro, ro + 32)
                            for h in range(2):
                                pq = ps.tile([C, 512], f32, tag="zh",
                                             bufs=2,
                                             name=f"p{which}{i}_{h}")
                                nc.tensor.matmul(
                                    pq[:, :], wt_r, fr[:, ts(h, 512)],
                                    start=True, stop=True)
                                nc.vector.tensor_copy(pqs[rs, ts(h, 512)],
                                                      pq[rs, :])
                        box[("pqs", which)] = pqs
                    return s_proj

                def mk_sq(which):
                    bbc = bb1_sb if which == "q" else bb2_sb
                    def s_sq():
                        pqs = box[("pqs", which)]
                        sq = sqp.tile([C, HW], bf16, tag="sq")
                        nc.vector.tensor_mul(sq[rw, :], pqs[rw, :],
                                             pqs[rw, :])
                        # sq2 = q^2 + 2b*q: folds the bias cross-term so
                        # nsq needs a single-group matmul (the two-group
                        # quadrant accumulation miscomputes on HW)
                        nc.vector.scalar_tensor_tensor(
                            out=sq[rw, :], in0=pqs[rw, :],
                            scalar=bbc[rw, :], in1=sq[rw, :],
                            op0=mybir.AluOpType.mult,
                            op1=mybir.AluOpType.add)
                        box[("sq", which)] = sq
                    return s_sq

                def mk_nsq(which):
                    def s_nsq():
                        sq = box.pop(("sq", which))
                        nsq = [ps.tile([C, 512], f32, tag="zh", bufs=2,
                                       name=f"nsq{which}{quad}_{r0}_{h}")
                               for h in range(2)]
                        for h in range(2):
                            nc.tensor.matmul(nsq[h][rw, :],
                                             blk_r[rw, rw],
                                             sq[rw, ts(h, 512)],
                                             start=True, stop=True,
                                             tile_position=(r0, r0))
                        box[("nsq", which)] = nsq
                    return s_nsq

                def mk_ln(which):
                    nb = nb1_sb if which == "q" else nb2_sb
                    def s_ln():
                        nsq = box.pop(("nsq", which))
                        if dbg["on"] and which == "q" and quad == 0:
                            for h in range(2):
                                dnt = lep.tile([C, HW], f32, tag="le")
                                nc.vector.memset(dnt[rw, ts(h, 512)], 0.0)
                                nc.vector.tensor_copy(dnt[rw, ts(h, 512)],
                                                      nsq[h][rw, :])
                                nc.sync.dma_start(dn_d[0, rw, ts(h, 512)],
                                                  dnt[rw, ts(h, 512)])
                        lnv = lep.tile([C, HW], f32, tag="le")
                        for h in range(2):
                            nc.scalar.activation(lnv[rw, ts(h, 512)],
                                                 nsq[h][rw, :], AF.Ln,
                                                 bias=nb[rw, :])
                        box[("ln", which)] = lnv
                    return s_ln

                def mk_binv(which):
                    def s_binv():
                        lnv = box.pop(("ln", which))
                        bi = lep.tile([C, HW], f32, tag="le")
                        nc.scalar.activation(bi[rw, :], lnv[rw, :], AF.Exp,
                                             scale=-0.5)
                        box[("bi", which)] = bi
                    return s_binv

                def mk_aff(which):
                    bias = b1_sb if which == "q" else b2_sb
                    def s_aff():
                        dst = qtn[quad] if which == "q" else ktn[quad]
                        pqs = box.pop(("pqs", which))
                        bi = box.pop(("bi", which))
                        # affine_mul_reduce (custom DVE) miscomputes at
                        # base_partition != 0; scalar_tensor_tensor is
                        # the offset-safe equivalent: (pqs + b) * binv
                        nc.vector.scalar_tensor_tensor(
                            out=dst[rw, :], in0=pqs[rw, :],
                            scalar=bias[rw, :], in1=bi[rw, :],
                            op0=mybir.AluOpType.add,
                            op1=mybir.AluOpType.mult)
                    return s_aff

                return {
                    "dma_f1": dma_f1, "dma_f2": dma_f2,
                    "proj_q": mk_proj("q"), "sq_q": mk_sq("q"),
                    "nsq_q": mk_nsq("q"), "ln_q": mk_ln("q"),
                    "binv_q": mk_binv("q"), "aff_q": mk_aff("q"),
                    "proj_k": mk_proj("k"), "sq_k": mk_sq("k"),
                    "nsq_k": mk_nsq("k"), "ln_k": mk_ln("k"),
                    "binv_k": mk_binv("k"), "aff_k": mk_aff("k"),
                }

            def transp_stage(i, tag):
                def fn():
                    f2r = f2_sbs[i][:]
                    x2 = x2tp.tile([C, HW], bf16, tag="x2t", name=f"x2t{i}")
                    if tag == "av":
                        pt = ps.tile([C, HW], f32, tag=tag, bufs=1,
                                     name=f"pt{i}")
                        for j in range(8):
                            nc.tensor.transpose(
                                pt[:, ts(j, C)].bitcast(f32r),
                                f2r[:, ts(j, C)], id_r)
                        nc.vector.tensor_copy(x2[:], pt[:])
                    else:
                        for h in range(2):
                            pt = ps.tile([C, 512], f32, tag="zh", bufs=2,
                                         name=f"pt{i}_{h}")
                            for j in range(4):
                                nc.tensor.transpose(
                                    pt[:, ts(j, C)].bitcast(f32r),
                                    f2r[:, ts(4 * h + j, C)], id_r)
                            nc.vector.tensor_copy(x2[:, ts(h, 512)], pt[:])
                    x2t_sbs[i] = x2
                return fn

            # ---------- attention streams ----------
            def mk_s(quad, row, j, nm):
                s = ps.tile([C, HW], f32, tag="s", bufs=2, name=nm)
                for h in range(2):
                    nc.tensor.matmul(s[:, ts(h, 512)],
                                     ktn[quad][row:row + 32, ts(j, C)],
                                     qtn[quad][row:row + 32, ts(h, 512)],
                                     start=True, stop=True,
                                     tile_position=(row, 0))
                return s

            def push_zav(i):
                """Deferred work for instance i: Z add-tree (DVE) + one
                ones-matmul, bz broadcast, AV matmuls, DVE tail."""
                st = {}

                def tree_add(dst_key, a_key, b_key):
                    def fn():
                        t = ztp.tile([C, HW], bf16, tag="zt",
                                     name=f"zt{i}_{dst_key}")
                        ea = (e_tiles[(i, a_key)] if isinstance(a_key, int)
                              else st[a_key])
                        eb = (e_tiles[(i, b_key)] if isinstance(b_key, int)
                              else st[b_key])
                        nc.vector.tensor_add(t[:], ea[:], eb[:])
                        st[dst_key] = t
                    return (1, fn)

                def z_mm():
                    z = [ps.tile([1, 512], f32, tag="zh", bufs=2,
                                 name=f"z{i}_{h}") for h in range(2)]
                    for h in range(2):
                        nc.tensor.matmul(z[h][0:1, :], ones_bf[:],
                                         st["s07"][:, ts(h, 512)],
                                         start=True, stop=True)
                    st["z"] = z

                def chain():
                    # z evac -> bz broadcast -> reciprocal (per half)
                    zh = st.pop("z")
                    zs = zsp.tile([1, HW], f32r, tag="zs", name=f"zs{i}")
                    for h in range(2):
                        nc.vector.tensor_copy(zs[0:1, ts(h, 512)],
                                              zh[h][0:1, :])
                    zsb[i] = zs
                    if dbg["on"]:
                        nc.sync.dma_start(dz_d[i, :, :],
                                          zs[0:1, :].bitcast(f32))
                    bzi = bzp.tile([C, HW], f32, tag="bzi", name=f"bzi{i}")
                    for h in range(2):
                        bz = ps.tile([C, 512], f32, tag="zh", bufs=2,
                                     name=f"bz{i}_{h}")
                        nc.tensor.matmul(bz[:, :], onesrow_r,
                                         zs[0:1, ts(h, 512)],
                                         start=True, stop=True)
                        nc.vector.reciprocal_approx_fast(bzi[:, ts(h, 512)],
                                                         bz[:, :])
                    st["bzi"] = bzi

                def av_mm(j):
                    def fn():
                        if j == 0:
                            st["av"] = ps.tile([C, HW], f32, tag="av",
                                               bufs=1, name=f"av{i}")
                        e = e_tiles.pop((i, j))
                        for h in range(2):
                            nc.tensor.matmul(st["av"][:, ts(h, 512)],
                                             x2t_sbs[i][:, ts(j, C)],
                                             e[:, ts(h, 512)],
                                             start=(j == 0), stop=(j == 7))
                    return (1, fn)

                def tail():
                    t1 = t1p.tile([C, HW], f32, tag="t1", name=f"t1_{i}")
                    nc.vector.tensor_mul(t1[:], st.pop("av")[:],
                                         st.pop("bzi")[:])
                    r = rp.tile([C, HW], f32, tag="r", name=f"r{i}")
                    nc.vector.affine_then_add(r[:], f2_sbs.pop(i)[:], t1[:],
                                              scale=1.0, bias=t2_sb[:])
                    r_sbs[i] = r
                    if dbg["on"]:
                        for h in range(2):
                            nc.sync.dma_start(dr_d[i, :, ts(h, 512)],
                                              r[:, ts(h, 512)])
                    scr = scrp.tile([C, HW], f32, tag="scr")
                    ssq = colp.tile([C, 1], f32, tag="ssq", bufs=10,
                                    name=f"ssq{i}")
                    nc.vector.affine_mul_reduce(out=scr[:], accum_out=ssq[:],
                                                in0=r[:], in1=r[:],
                                                scale=1.0, bias=0.0)
                    ssq_cols[i] = ssq

                pe_fifo.append(tree_add("s01", 0, 1))
                pe_fifo.append(tree_add("s23", 2, 3))
                pe_fifo.append(tree_add("s45", 4, 5))
                pe_fifo.append(tree_add("s67", 6, 7))
                pe_fifo.append(tree_add("s03", "s01", "s23"))
                pe_fifo.append(tree_add("s47", "s45", "s67"))
                pe_fifo.append(tree_add("s07", "s03", "s47"))
                pe_fifo.append((1, z_mm))
                pe_fifo.append((1, chain))
                for j in range(8):
                    pe_fifo.append(av_mm(j))
                pe_fifo.append((2, tail))

            def phase_c_stages(insts):
                n = len(insts)
                box = {}

                def s_gather():
                    pg = ps.tile([1, n], f32, tag="zh", bufs=2,
                                 name=f"pg{insts[0]}")
                    for k, i in enumerate(insts):
                        nc.tensor.matmul(pg[0:1, k:k + 1], ones_f[:],
                                         ssq_cols.pop(i)[:],
                                         start=True, stop=True)
                    gl = gp.tile([1, n], f32, tag="gl")
                    nc.scalar.activation(gl[0:1, :], pg[0:1, :], AF.Ln,
                                         scale=1.0 / 64.0, bias=gbias_sb[:])
                    g = gp.tile([1, n], f32, tag="g")
                    nc.scalar.activation(g[0:1, :], gl[0:1, :], AF.Exp,
                                         scale=-0.5)
                    box["g"] = g

                def s_bcast():
                    pgc = ps.tile([C, n], f32, tag="zh", bufs=2,
                                  name=f"pgc{insts[0]}")
                    for k in range(n):
                        nc.tensor.matmul(pgc[:, k:k + 1], onesrow_f,
                                         box["g"][0:1, k:k + 1],
                                         start=True, stop=True)
                    gc = gp.tile([C, n], f32, tag="gc")
                    nc.vector.tensor_copy(gc[:], pgc[:])
                    box["gc"] = gc

                def mk_out(k, i):
                    def fn():
                        rr = r_sbs.pop(i)
                        o = op.tile([C, HW], f32, tag="o")
                        for h in range(2):
                            sl = ts(h, 512)
                            nc.vector.tensor_scalar_mul(
                                o[:, sl], rr[:, sl], box["gc"][:, k:k + 1])
                            nc.sync.dma_start(out_d[i, :, sl], o[:, sl])
                    return fn

                sts = [s_gather, s_bcast]
                for k, i in enumerate(insts):
                    sts.append(mk_out(k, i))
                return sts

            def pair_stream(quad, iA, iB, rowA, rowB, staged, inst_major):
                """16-step exp stream for (iA, iB). staged[step] closures
                pace norm/transpose/phase_c work; fifo drains fill PE."""

                def do_step(exp_src, i, j):
                    e = ep.tile([C, HW], bf16, tag="e", name=f"e{i}_{j}")
                    nc.scalar.activation(e[:], exp_src[:], AF.Exp,
                                         scale=30.0)
                    e_tiles[(i, j)] = e

                if inst_major:
                    order = ([(iA, rowA, j) for j in range(8)]
                             + [(iB, rowB, j) for j in range(8)])
                else:
                    order = []
                    for j in range(8):
                        order.append((iA, rowA, j))
                        order.append((iB, rowB, j))

                live = [mk_s(quad, r_, j_, f"s{i_}_{j_}")
                        for (i_, r_, j_) in order[:2]]
                for step, (i, row, j) in enumerate(order):
                    do_step(live.pop(0), i, j)
                    for fn in staged.get(step, ()):
                        fn()
                    drain(3)
                    if step + 2 < 16:
                        i2, r2, j2 = order[step + 2]
                        live.append(mk_s(quad, r2, j2, f"s{i2}_{j2}"))

            def finish_early():
                """Truncated debug builds: flush fifo, zero the output."""
                while pe_fifo:
                    drain(100)
                zt = op.tile([C, HW], f32, tag="o")
                nc.vector.memset(zt[:], 0.0)
                for i in range(ipc):
                    for h in range(2):
                        nc.sync.dma_start(out_d[i, :, ts(h, 512)],
                                          zt[:, ts(h, 512)])
                if debug:
                    for i in range(ipc):
                        if i not in zsb:
                            nc.sync.dma_start(dz_d[i, :, :],
                                              zt[0:1, :])
                        if i not in r_sbs:
                            for h in range(2):
                                nc.sync.dma_start(dr_d[i, :, ts(h, 512)],
                                                  zt[:, ts(h, 512)])
                    dbgp2 = ctx.enter_context(
                        tc.tile_pool(name="dbgp2", bufs=2))
                    for quad in sorted(qtn):
                        dqt = dbgp2.tile([C, HW], f32, tag="dbg")
                        nc.vector.tensor_copy(dqt[:], qtn[quad][:])
                        for h in range(2):
                            nc.sync.dma_start(dq_d[quad, :, ts(h, 512)],
                                              dqt[:, ts(h, 512)])
                        dkt = dbgp2.tile([C, HW], f32, tag="dbg")
                        nc.vector.tensor_copy(dkt[:], ktn[quad][:])
                        for h in range(2):
                            nc.sync.dma_start(dk_d[quad, :, ts(h, 512)],
                                              dkt[:, ts(h, 512)])

            # ================= orchestration =================
            # instance-granular norms for 0 and 1 (fast startup)
            n0 = norm_stages([0], 0, 0)
            n1 = norm_stages([1], 0, 32)
            for key in ("dma_f1", "dma_f2"):
                n0[key]()
                n1[key]()
            for key in ("proj_q", "sq_q", "nsq_q", "ln_q", "binv_q", "aff_q",
                        "proj_k", "sq_k", "nsq_k", "ln_k", "binv_k",
                        "aff_k"):
                n0[key]()
                n1[key]()
            pe_fifo.append((3, transp_stage(0, "av")))
            pe_fifo.append((3, transp_stage(1, "av")))
            if stop_after == "norm01":
                finish_early()

            # pair (0,1): instance-major; norm (2,3) + quad1 f1-DMA staged
            n23 = norm_stages([2, 3], 0, 64)
            ng1 = norm_stages([4, 5, 6, 7], 1, 0)
            staged01 = {
                0: [n23["dma_f1"], n23["dma_f2"]],
                4: [n23["proj_q"]],
                5: ([n23["sq_q"], ng1["dma_f1"]]
                    if stop_after not in ("pair01", "pair23")
                    else [n23["sq_q"]]),
                6: [n23["nsq_q"]], 7: [n23["ln_q"]], 8: [n23["binv_q"]],
                9: [n23["aff_q"], n23["proj_k"]], 10: [n23["sq_k"]],
                11: [n23["nsq_k"]], 12: [n23["ln_k"]], 13: [n23["binv_k"]],
                14: [n23["aff_k"]],
                15: [transp_stage(2, "av"), transp_stage(3, "av")],
            }
            if stop_after != "norm01":
                pair_stream(0, 0, 1, 0, 32, staged01, inst_major=True)
                push_zav(0)
                push_zav(1)
            if stop_after == "pair01":
                finish_early()

            # pair (2,3): j-major; quad1 norm staged inside
            if stop_after in ("pair01", "pair23"):
                staged23 = {}
            else:
                staged23 = {
                    0: [ng1["dma_f2"]],
                    4: [ng1["proj_q"]], 5: [ng1["sq_q"]],
                    6: [ng1["nsq_q"]], 7: [ng1["ln_q"]],
                    8: [ng1["binv_q"]], 9: [ng1["aff_q"]],
                    11: [ng1["proj_k"]], 12: [ng1["sq_k"]],
                    13: [ng1["nsq_k"]], 14: [ng1["ln_k"]],
                    15: [ng1["binv_k"], ng1["aff_k"]],
                }
            if stop_after not in ("norm01", "pair01"):
                pair_stream(0, 2, 3, 64, 96, staged23, inst_major=False)
                push_zav(2)
                push_zav(3)
            if stop_after == "pair23":
                finish_early()

            # pair (4,5): quad1 transposes staged
            staged45 = {
                1: [transp_stage(4, "zb")], 4: [transp_stage(5, "zb")],
                7: [transp_stage(6, "zb")], 10: [transp_stage(7, "zb")],
            }
            _done = stop_after in ("norm01", "pair01", "pair23")
            if not _done:
                pair_stream(1, 4, 5, 0, 32, staged45, inst_major=False)
                push_zav(4)
                push_zav(5)
            if stop_after == "pair45":
                finish_early()

            # pair (6,7): phase_c for quad0 staged
            _done = stop_after in ("norm01", "pair01", "pair23", "pair45")
            if not _done:
                pc0 = phase_c_stages([0, 1, 2, 3])
                staged67 = {2: [pc0[0]], 4: [pc0[1]], 6: [pc0[2]],
                            8: [pc0[3]], 10: [pc0[4]], 12: [pc0[5]]}
                pair_stream(1, 6, 7, 64, 96, staged67, inst_major=False)
                push_zav(6)
                push_zav(7)
            if stop_after is None:
                # flush + final epilogue
                while pe_fifo:
                    drain(100)
                if debug:
                    dbgp = ctx.enter_context(
                        tc.tile_pool(name="dbgp", bufs=2))
                    for quad in (0, 1):
                        dqt = dbgp.tile([C, HW], f32, tag="dbg")
                        nc.vector.tensor_copy(dqt[:], qtn[quad][:])
                        for h in range(2):
                            nc.sync.dma_start(dq_d[quad, :, ts(h, 512)],
                                              dqt[:, ts(h, 512)])
                        dkt = dbgp.tile([C, HW], f32, tag="dbg")
                        nc.vector.tensor_copy(dkt[:], ktn[quad][:])
                        for h in range(2):
                            nc.sync.dma_start(dk_d[quad, :, ts(h, 512)],
                                              dkt[:, ts(h, 512)])

                for fn in phase_c_stages([4, 5, 6, 7]):
                    fn()

    nc.compile()
    return nc


def make_in_maps(inputs, ncores=NCORES):
    f_list1 = np.asarray(inputs["f_list1"], dtype=np.float32)
    f_list2 = np.asarray(inputs["f_list2"], dtype=np.float32)
    t_pos1 = np.asarray(inputs["t_pos1"], dtype=np.float32).reshape(C)
    t_pos2 = np.asarray(inputs["t_pos2"], dtype=np.float32).reshape(C)
    W_qk_w = np.asarray(inputs["W_qk_w"], dtype=np.float32)
    W_qk_b = np.asarray(inputs["W_qk_b"], dtype=np.float32)

    # fold t_pos into the projection biases: q = W @ (x + t1) + b
    b1v = (W_qk_w @ t_pos1 + W_qk_b).astype(np.float32)        # (32,)
    b2v = (W_qk_w @ t_pos2 + W_qk_b).astype(np.float32)
    b1 = np.tile(b1v.reshape(32, 1), (4, 1))                   # (128, 1)
    b2 = np.tile(b2v.reshape(32, 1), (4, 1))
    wt = np.ascontiguousarray(np.tile(W_qk_w.T, (1, 4)))       # (128, 128)
    eye4 = np.eye(4, dtype=np.float32)
    blk = np.kron(eye4, np.ones((32, 32), dtype=np.float32))   # (128, 128)
    bb1 = np.tile((2.0 * b1v).reshape(32, 1), (4, 1))          # (128, 1)
    bb2 = np.tile((2.0 * b2v).reshape(32, 1), (4, 1))
    nb1 = np.full((C, 1), float(np.sum(b1v * b1v)), np.float32)
    nb2 = np.full((C, 1), float(np.sum(b2v * b2v)), np.float32)
    ident = np.eye(C, dtype=np.float32)
    t2x2 = (2.0 * t_pos2).astype(np.float32).reshape(C, 1)

    f1 = np.ascontiguousarray(f_list1.reshape(NI, C, HW))
    f2 = np.ascontiguousarray(f_list2.reshape(NI, C, HW))

    in_maps = []
    for c in range(ncores):
        sl = slice(c * IPC, (c + 1) * IPC)
        in_maps.append({
            "f1": np.ascontiguousarray(f1[sl]),
            "f2": np.ascontiguousarray(f2[sl]),
            "wt": wt, "blk": blk, "bb1": bb1, "bb2": bb2, "ident": ident,
            "b1": b1, "b2": b2, "nb1": nb1, "nb2": nb2, "t2x2": t2x2,
        })
    return in_maps


def kernel(**inputs) -> np.ndarray:
    from concourse import bass_utils

    if "nc" not in _CACHE:
        _CACHE["nc"] = _build()
    nc = _CACHE["nc"]

    in_maps = make_in_maps(inputs)
    res = bass_utils.run_bass_kernel_spmd(nc, in_maps,
                                          core_ids=list(range(NCORES)))
    out = np.empty((NI, C, HW), dtype=np.float32)
    for c in range(NCORES):
        out[c * IPC:(c + 1) * IPC] = res.results[c]["out"]
    return out.reshape(NI, C, H, W)
